# revision 18
# baseline (speedup 1.0000x reference)
"""Trainium2 Bass kernel for nn_CentralPathNet.

Pipeline per core (data-parallel over batch, 4096 rows/core):
  1. MLP 512->256->256->128 in "transposed" orientation (batch on the
     moving free dim, weights stationary), fp16.
  2. Head matmul with host-side pre-symmetrized weights: emits
     Ahalf = (Ms - eps*I)/2 for both matrix kinds directly (plus y),
     in natural [batch, chan] layout.
  3. Layout shuffle natural -> "stack4" ([4 matrices x 32 rows] on
     partitions, [groups x 32 cols] on free) via a DRAM round-trip.
  4. Matrix-sign Newton-type cubic iteration (optimally scaled) on the
     32x32 PE sub-tiles: S = sign(Ms - eps*I).
  5. P = S*Ahalf + Ahalf  (= (A + |A|)/2 = relu-projection); host adds
     eps*I and unscrambles.

Everything needed at grading time is in this file (shapes hardcoded).
"""

import numpy as np
from contextlib import ExitStack

import concourse.bass as bass
import concourse.tile as tile
from concourse import bacc, mybir
from concourse.bass_utils import run_bass_kernel_spmd

# ---------------- problem constants ----------------
NCORES = 8
BTOT, D = 32768, 512
H1, H2, H3 = 256, 256, 128
NN = 32                      # matrix dim
EPS = 1e-4
GS = 0.63                    # global spectral normalization scale
NCH = 2176                   # padded head channels: 1024 X | 1024 S | 64 y | 64 pad

BPC = BTOT // NCORES         # 4096 rows per core
BB = 1024                    # rows per pass
NPASS = BPC // BB            # 4
NBT = BB // 128              # 8 b-tiles per pass
GPP = (2 * BB) // 4          # stack groups per pass (both kinds) = 512
WQ = 8                       # groups per wave (32 matrices)
NWAVE = GPP // WQ            # 64 waves/pass
FW = 32 * WQ                 # free width per wave = 256

FP16 = mybir.dt.float16
FP32 = mybir.dt.float32
AOP = mybir.AluOpType
AF = mybir.ActivationFunctionType


def _coeffs():
    """Optimally-scaled cubic sign-iteration schedule on [l0, u0]."""
    l, u = 7e-4, 0.95
    cs = []
    for _ in range(16):
        c = u * u + u * l + l * l
        b = -1.5 * np.sqrt(3.0) * c ** -1.5
        a = -b * c
        xs = np.linspace(l, u, 40001)
        p = a * xs + b * xs ** 3
        cs.append((float(a), float(b)))
        l, u = float(p.min()), float(p.max())
        if 1.0 - l < 2e-6:
            break
    # fold global normalization X0 = (2/GS)*Ahalf into the first step
    c0 = 2.0 / GS
    a0, b0 = cs[0]
    cs[0] = (a0 * c0, b0 * c0 ** 3)
    return cs


COEFFS = _coeffs()
NSTEP = len(COEFFS)


# ---------------- device program ----------------
def build_program(bpc=BPC, bb=BB, nstep=NSTEP, wq=WQ, use_for_i=True,
                  debug_ast=False):
    npass = bpc // bb
    nbt = bb // 128
    gpp = (2 * bb) // 4
    nwave = gpp // wq
    fw = 32 * wq
    coeffs = COEFFS[:nstep]

    nc = bacc.Bacc("TRN2", target_bir_lowering=False, debug=False,
                   num_devices=NCORES)

    xT = nc.dram_tensor("xT", [D, bpc], FP16, kind="ExternalInput").ap()
    w1 = nc.dram_tensor("w1", [D, H1], FP16, kind="ExternalInput").ap()
    w2 = nc.dram_tensor("w2", [H1, H2], FP16, kind="ExternalInput").ap()
    w3 = nc.dram_tensor("w3", [H2, H3], FP16, kind="ExternalInput").ap()
    b1 = nc.dram_tensor("b1", [H1, 1], FP32, kind="ExternalInput").ap()
    b2 = nc.dram_tensor("b2", [H2, 1], FP32, kind="ExternalInput").ap()
    b3 = nc.dram_tensor("b3", [H3, 1], FP32, kind="ExternalInput").ap()
    whs = nc.dram_tensor("whs", [H3, NCH], FP16, kind="ExternalInput").ap()
    brow = nc.dram_tensor("brow", [128, NCH], FP32, kind="ExternalInput").ap()
    isc = nc.dram_tensor("isc", [128, fw * nstep], FP32, kind="ExternalInput").ap()

    pout = nc.dram_tensor("P", [128, npass * 16 * bb], FP32, kind="ExternalOutput").ap()
    yout = nc.dram_tensor("y", [bpc, 64], FP32, kind="ExternalOutput").ap()
    astdbg = (nc.dram_tensor("astdbg", [128, 16 * bb], FP32,
                             kind="ExternalOutput").ap() if debug_ast else None)
    stepdbg = (nc.dram_tensor("stepdbg", [nstep, 128, 16 * bb], FP32,
                              kind="ExternalOutput").ap() if debug_ast else None)

    with tile.TileContext(nc) as tc:
        with ExitStack() as ctx:
            _body(ctx, tc, locals())
    nc.compile()
    return nc


def _body(ctx, tc, v):
    nc = tc.nc
    xT, w1, w2, w3, whs, brow, isc = (v[k] for k in
                                      ("xT", "w1", "w2", "w3", "whs", "brow", "isc"))
    b1, b2, b3 = v["b1"], v["b2"], v["b3"]
    pout, yout = v["pout"], v["yout"]
    bb, npass, nbt = v["bb"], v["npass"], v["nbt"]
    nwave, wq, fw, nstep = v["nwave"], v["wq"], v["fw"], v["nstep"]
    coeffs = v["coeffs"]
    use_for_i = v["use_for_i"]

    consts = ctx.enter_context(tc.tile_pool(name="consts", bufs=1))
    big = ctx.enter_context(tc.tile_pool(name="big", bufs=1))
    natp = ctx.enter_context(tc.tile_pool(name="natp", bufs=3))
    natd = ctx.enter_context(tc.tile_pool(name="natd", bufs=1, space="DRAM"))
    mlpps = ctx.enter_context(tc.tile_pool(name="mlpps", bufs=2, space="PSUM"))
    itps = ctx.enter_context(tc.tile_pool(name="itps", bufs=2, space="PSUM"))
    itv = ctx.enter_context(tc.tile_pool(name="itv", bufs=3))
    yb = ctx.enter_context(tc.tile_pool(name="yb", bufs=2))
    nat = natd.tile([bb, 2048], FP32)

    # ---- load constants to SBUF ----
    w1s = consts.tile([128, 4 * H1], FP16)          # kc-chunks side by side
    for kc in range(4):
        nc.sync.dma_start(w1s[:, kc * H1:(kc + 1) * H1], w1[kc * 128:(kc + 1) * 128, :])
    w2s = consts.tile([128, 2 * H2], FP16)
    for kc in range(2):
        nc.sync.dma_start(w2s[:, kc * H2:(kc + 1) * H2], w2[kc * 128:(kc + 1) * 128, :])
    w3s = consts.tile([128, 2 * H3], FP16)
    for kc in range(2):
        nc.sync.dma_start(w3s[:, kc * H3:(kc + 1) * H3], w3[kc * 128:(kc + 1) * 128, :])
    b1s = consts.tile([128, 2], FP32)
    nc.sync.dma_start(b1s[:, 0:1], b1[0:128, :])
    nc.sync.dma_start(b1s[:, 1:2], b1[128:256, :])
    b2s = consts.tile([128, 2], FP32)
    nc.sync.dma_start(b2s[:, 0:1], b2[0:128, :])
    nc.sync.dma_start(b2s[:, 1:2], b2[128:256, :])
    b3s = consts.tile([128, 1], FP32)
    nc.sync.dma_start(b3s[:], b3[:, :])
    whss = consts.tile([128, NCH], FP16)
    nc.sync.dma_start(whss[:], whs[:, :])
    brows = consts.tile([128, NCH], FP32)
    nc.sync.dma_start(brows[:], brow[:, :])
    iscs = consts.tile([128, fw * nstep], FP32)
    nc.sync.dma_start(iscs[:], isc[:, :])

    xts = big.tile([128, 4 * bb], FP16)
    h1t = big.tile([128, 2 * bb], FP16)
    h2t = big.tile([128, 2 * bb], FP16)
    h3t = big.tile([128, bb], FP16)
    ast = big.tile([128, 16 * bb], FP32)            # Ahalf stack4: X | S

    cw = min(512, bb)                                # batch chunk width
    nbc = bb // cw

    for p in range(npass):
        b0 = p * bb
        # ---- load xT slice ----
        for dc in range(4):
            nc.sync.dma_start(xts[:, dc * bb:(dc + 1) * bb],
                              xT[dc * 128:(dc + 1) * 128, b0:b0 + bb])
        # ---- L1: h1T = relu(W1^T xT + b1) ----
        for jc in range(2):
            for bc in range(nbc):
                ps = mlpps.tile([128, cw], FP32, tag="mlp")
                for kc in range(4):
                    nc.tensor.matmul(
                        ps[:], w1s[:, kc * H1 + jc * 128: kc * H1 + jc * 128 + 128],
                        xts[:, kc * bb + bc * cw: kc * bb + bc * cw + cw],
                        start=(kc == 0), stop=(kc == 3))
                nc.scalar.activation(h1t[:, jc * bb + bc * cw: jc * bb + bc * cw + cw],
                                     ps[:], AF.Relu, bias=b1s[:, jc:jc + 1])
        # ---- L2 ----
        for jc in range(2):
            for bc in range(nbc):
                ps = mlpps.tile([128, cw], FP32, tag="mlp")
                for kc in range(2):
                    nc.tensor.matmul(
                        ps[:], w2s[:, kc * H2 + jc * 128: kc * H2 + jc * 128 + 128],
                        h1t[:, kc * bb + bc * cw: kc * bb + bc * cw + cw],
                        start=(kc == 0), stop=(kc == 1))
                nc.scalar.activation(h2t[:, jc * bb + bc * cw: jc * bb + bc * cw + cw],
                                     ps[:], AF.Relu, bias=b2s[:, jc:jc + 1])
        # ---- L3 ----
        for bc in range(nbc):
            ps = mlpps.tile([128, cw], FP32, tag="mlp")
            for kc in range(2):
                nc.tensor.matmul(ps[:], w3s[:, kc * H3: kc * H3 + 128],
                                 h2t[:, kc * bb + bc * cw: kc * bb + bc * cw + cw],
                                 start=(kc == 0), stop=(kc == 1))
            nc.scalar.activation(h3t[:, bc * cw:(bc + 1) * cw], ps[:], AF.Relu,
                                 bias=b3s[:])
        # ---- head + natural evac + DRAM scratch ----
        for bt in range(nbt):
            hb = bt * 128
            natt = natp.tile([128, 2048], FP32, tag="nat")
            for cc in range(4):
                ps = mlpps.tile([128, 512], FP32, tag="mlp")
                nc.tensor.matmul(ps[:], h3t[:, hb:hb + 128],
                                 whss[:, cc * 512:(cc + 1) * 512],
                                 start=True, stop=True)
                nc.vector.scalar_tensor_tensor(
                    natt[:, cc * 512:(cc + 1) * 512], ps[:], 1.0,
                    brows[:, cc * 512:(cc + 1) * 512], AOP.mult, AOP.add)
            # y channels (2048..2112)
            ps = mlpps.tile([128, 512], FP32, tag="mlp")
            nc.tensor.matmul(ps[:, 0:128], h3t[:, hb:hb + 128], whss[:, 2048:2176],
                             start=True, stop=True)
            ybuf = yb.tile([128, 64], FP32)
            nc.vector.scalar_tensor_tensor(ybuf[:], ps[:, 0:64], 1.0,
                                           brows[:, 2048:2112], AOP.mult, AOP.add)
            nc.sync.dma_start(yout[b0 + hb: b0 + hb + 128, :], ybuf[:])
            nc.sync.dma_start(nat[hb:hb + 128, :], natt[:])
        # ---- shuffle: DRAM natural -> SBUF stack4 ----
        # ast[32a+k, kind*8*bb + 32g + j] = nat[4g+a, kind*1024 + 32k + j]
        natr = nat[:].rearrange("(g a) (kind k j) -> kind a k g j",
                                a=4, kind=2, k=32, j=32)
        astr = ast[:].rearrange("(a k) (kind g j) -> kind a k g j",
                                a=4, k=32, kind=2, j=32)
        for kind in range(2):
            for aa in range(4):
                nc.sync.dma_start(astr[kind][aa], natr[kind][aa])
        if v.get("astdbg") is not None and p == 0:
            nc.sync.dma_start(v["astdbg"][:, :], ast[:])

        # ---- sign iteration (stack4, diagonal 32x32 PE tiles) ----
        def wave_body(base_off):
            """base_off: element offset of the wave window in ast free dim."""
            a0 = itv.tile([128, fw], FP32, tag="a0")
            nc.vector.tensor_copy(a0[:], ast[:, bass.ds(base_off, fw)])
            cur = a0
            for s, (a_s, b_s) in enumerate(coeffs):
                csrc = cur

                def src(a, q, _c=csrc):
                    return _c[32 * a:32 * a + 32, q * 32:q * 32 + 32]
                zp = itps.tile([128, fw], FP32, tag="zp")
                for q in range(wq):
                    for a in range(4):
                        st = src(a, q)
                        nc.tensor.matmul(zp[32 * a:32 * a + 32, q * 32:q * 32 + 32],
                                         st, st, start=True, stop=True,
                                         tile_position=(32 * a, 32 * a))
                vb = itv.tile([128, fw], FP32, tag="vb")
                nc.vector.scalar_tensor_tensor(vb[:], zp[:], b_s,
                                               iscs[:, s * fw:(s + 1) * fw],
                                               AOP.mult, AOP.add)
                xp = itps.tile([128, fw], FP32, tag="xp")
                for q in range(wq):
                    for a in range(4):
                        nc.tensor.matmul(xp[32 * a:32 * a + 32, q * 32:q * 32 + 32],
                                         vb[32 * a:32 * a + 32, q * 32:q * 32 + 32],
                                         src(a, q), start=True, stop=True,
                                         tile_position=(32 * a, 32 * a))
                xn = itv.tile([128, fw], FP32, tag="xc")
                nc.vector.tensor_copy(xn[:], xp[:])
                cur = xn
                if v.get("stepdbg") is not None and p == 0 and isinstance(base_off, int):
                    nc.sync.dma_start(
                        v["stepdbg"][s, :, base_off:base_off + fw], xn[:])
            # final: P = S*Ahalf + Ahalf
            pp = itps.tile([128, fw], FP32, tag="zp")
            for q in range(wq):
                for a in range(4):
                    nc.tensor.matmul(pp[32 * a:32 * a + 32, q * 32:q * 32 + 32],
                                     cur[32 * a:32 * a + 32, q * 32:q * 32 + 32],
                                     a0[32 * a:32 * a + 32, q * 32:q * 32 + 32],
                                     start=True, stop=True,
                                     tile_position=(32 * a, 32 * a))
            pw = itv.tile([128, fw], FP32, tag="pw")
            nc.vector.tensor_add(pw[:], pp[:], a0[:])
            nc.sync.dma_start(pout[:, bass.ds(p * 16 * bb + base_off, fw)], pw[:])

        if use_for_i:
            with tc.For_i(0, nwave // 2, 1,
                          hint_engines=(mybir.EngineType.PE,
                                        mybir.EngineType.DVE,
                                        mybir.EngineType.SP)) as t:
                wave_body(t * (2 * fw))
                wave_body(t * (2 * fw) + fw)
        else:
            for wv in range(nwave):
                wave_body(wv * fw)


# ---------------- host-side glue ----------------
def _host_prep(inputs):
    x = np.asarray(inputs["x"], dtype=np.float32)
    W1 = np.asarray(inputs["W1"]); W2 = np.asarray(inputs["W2"])
    W3 = np.asarray(inputs["W3"]); Wh = np.asarray(inputs["Wh"])
    bh = np.asarray(inputs["bh"])
    b1 = np.asarray(inputs["b1"]); b2 = np.asarray(inputs["b2"])
    b3 = np.asarray(inputs["b3"])

    n2 = NN * NN
    # symmetrized + halved head weights; -eps/2 diagonal via bias row
    k_idx = np.arange(n2) // NN
    j_idx = np.arange(n2) % NN
    swap = j_idx * NN + k_idx
    whs = np.zeros((H3, NCH), np.float32)
    browv = np.zeros((NCH,), np.float32)
    WhX = Wh[:, :n2]; bhX = bh[:n2]
    WhS_ = Wh[:, n2 + 64:]; bhS = bh[n2 + 64:]
    whs[:, :n2] = 0.25 * (WhX + WhX[:, swap])
    browv[:n2] = 0.25 * (bhX + bhX[swap])
    whs[:, n2:2 * n2] = 0.25 * (WhS_ + WhS_[:, swap])
    browv[n2:2 * n2] = 0.25 * (bhS + bhS[swap])
    diag = (k_idx == j_idx)
    browv[:n2][diag] -= EPS / 2
    browv[n2:2 * n2][diag] -= EPS / 2
    whs[:, 2048:2112] = Wh[:, n2:n2 + 64]
    browv[2048:2112] = bh[n2:n2 + 64]
    brow = np.broadcast_to(browv, (128, NCH)).copy()

    # per-step a_s * I patterns, replicated across the wave width
    isc = np.zeros((128, FW * NSTEP), np.float32)
    eye = np.zeros((128, 32), np.float32)
    for a in range(4):
        eye[32 * a:32 * a + 32, :] = np.eye(32)
    for s, (a_s, _) in enumerate(COEFFS):
        isc[:, s * FW:(s + 1) * FW] = np.tile(a_s * eye, (1, WQ))

    shared = {
        "w1": W1.astype(np.float16), "w2": W2.astype(np.float16),
        "w3": W3.astype(np.float16),
        "b1": b1.reshape(-1, 1).astype(np.float32),
        "b2": b2.reshape(-1, 1).astype(np.float32),
        "b3": b3.reshape(-1, 1).astype(np.float32),
        "whs": whs.astype(np.float16), "brow": brow.astype(np.float32),
        "isc": isc.astype(np.float32),
    }
    in_maps = []
    for c in range(NCORES):
        m = dict(shared)
        m["xT"] = np.ascontiguousarray(
            x[c * BPC:(c + 1) * BPC, :].T).astype(np.float16)
        in_maps.append(m)
    return in_maps


def _unscramble(res):
    """res: list of per-core dicts with 'P' [128, 65536] fp16, 'y' [4096,64]."""
    Xs, ys, Ss = [], [], []
    eye = EPS * np.eye(NN, dtype=np.float32)
    for c in range(NCORES):
        P = np.asarray(res[c]["P"]).astype(np.float32)
        arr = P.reshape(4, 32, NPASS, 2, BB // 4, 32)  # [a,k,pass,kind,g,j]
        arr = arr.transpose(2, 4, 0, 3, 1, 5)          # [pass,g,a,kind,k,j]
        arr = arr.reshape(BPC, 2, NN, NN)
        Xs.append(arr[:, 0] + eye)
        Ss.append(arr[:, 1] + eye)
        ys.append(np.asarray(res[c]["y"]).astype(np.float32))
    return (np.concatenate(Xs, 0), np.concatenate(ys, 0), np.concatenate(Ss, 0))


_NC_CACHE = {}


def kernel(**inputs):
    if "nc" not in _NC_CACHE:
        _NC_CACHE["nc"] = build_program()
    nc = _NC_CACHE["nc"]
    in_maps = _host_prep(inputs)
    out = run_bass_kernel_spmd(nc, in_maps, list(range(NCORES)))
    return _unscramble(out.results)


if __name__ == "__main__":
    import reference
    inputs = reference.setup_inputs()
    outs = kernel(**{k: np.asarray(v) for k, v in inputs.items()})
    print("kernel ran:", [o.shape for o in outs])
